# revision 27
# baseline (speedup 1.0000x reference)
"""Trainium2 Bass kernel for nn_DifferentiableSimulator.

Strategy (8 NeuronCores, B=8): one batch element per core, no collectives.

Host side (cheap, O(V+N)):
  - per-batch probe geometry: rotation, LUT bilinear interp (tiny)
  - per-batch voxel relevance sharding: keep voxels within CUT of the
    shank axis segment (+ probe radius).  Dropped voxels have weights
    <= e^{-CUT^2/4.5} relative to any weight that can influence an
    output pixel; empirically the output matches the dense reference
    to well under the harness tolerance.
  - lattice factorization: the 1000 contacts are a rigid 10x10x10 grid,
    so in the rotated frame  d2[n,v] = (x_i-wx_v)^2 + (y_j-wy_v)^2 +
    (z_k-wz_v)^2  with w = R^T (v - grid_center).  The soft-match weight
    matrix factorizes as W[n,v] = Wxy[(ij),v] * Wz[k,v]: only 110 gaussian
    columns per voxel instead of 1000.  Host ships the voxel features
    (fp16 hi/lo pairs so the fp16 matmul is ~fp32-exact: fp16 products are
    exact in the fp32 PSUM accumulator) and the 138 lattice columns.
  - contacts are reindexed m = k*128 + (iy*10+ix)  (28 dummy xy slots per
    z-layer with weight 0) so the per-z-layer weighted sums land exactly
    in contact-chunk layout with no transposes.

Device side (per core), phase 1 -- soft PRF match per 128-voxel chunk:
  one K=17 fp16 matmul -> xy/z gaussian exponents [128v, 138] in PSUM;
  ACT exp -> [Wxy | Wz] fp32; one DVE op forms WzE = Wz x [pol, ecc, 1]
  (broadcast APs); one fp32 matmul accumulates B[128ij, 30] =
  sum_v Wxy^T (Wz*E)  = all weighted sums, already contact-major.

Phase 2 -- separable splat, 96x96 window:
  phos_size = max(KSIG*m_inv, 1.0) == 1.0 identically for the reachable
  ecc range [0, 12] (KSIG*m_inv <= 0.447), so every gaussian has sigma
  exactly 1 px.  All splat centers lie in 128 +- 34.1 px (ecc <= 12),
  so only the [80,176) x [80,176) window of the 256x256 map is nonzero
  (>=12.9 px margin -> exp(-166), flushes to 0 in fp32 exactly like the
  reference).  sin/cos via a degree-9 odd polynomial on the DVE (no ACT
  table swaps; the only ACT table used is the Exp set).  Row factors
  (DVE) and col factors (Pool) are built as fp16 mega-tiles for 5 layers
  at a time, exp'd in two big ACT ops, weighted by wc via a broadcast
  multiply, and accumulated by 10 fp16 matmuls (K=128 contacts, N=96)
  into one [96,96] PSUM map; global max via the PE-transpose trick;
  scale; DMA the window out (borders are DMA'd from a zero tile early).

A PE warmup burst runs during startup to ramp the PE clock toward
2.4 GHz before phase 1.
"""
import math
from contextlib import ExitStack

import numpy as np

import concourse.bass as bass
import concourse.mybir as mybir
from concourse import tile
from concourse.bass_utils import run_bass_kernel_spmd

# ---- constants (must match the reference) ----
_CMAG_A = 0.75
_CMAG_B = 120.0
_CMAG_K = 17.3
_DEG2RAD = math.pi / 180.0
AMP = 100.0
_SPREAD = math.sqrt(AMP / 675.0)
VIEW_ANGLE = 90.0
MAP_SIZE = 256
SOFT_MATCH_SIGMA = 1.5

B = 8
NCC = 10                  # contact chunks = z-layers
NXY = 128                 # xy-lattice slots per layer (100 real + 28 dummy)
CUT = 7.0
XY_RAD = 1.8 * math.sqrt(2.0)
SE = MAP_SIZE / VIEW_ANGLE
KSIG = _SPREAD / 2.0 * SE
EXP_SCALE = 2.0 / (2.0 * SOFT_MATCH_SIGMA ** 2)   # 2/4.5
W0 = 80                   # splat window start (rows and cols)
WN = 96                   # splat window size

# sin(y) ~ y*(c0 + c1 y^2 + ... + c4 y^8) on [-pi, pi]; max err 1.7e-5
SIN_C = (9.99984590e-01, -1.66632589e-01, 8.31238590e-03,
         -1.93162309e-04, 2.17323611e-06)

f32 = mybir.dt.float32
f16 = mybir.dt.float16
i32 = mybir.dt.int32
AF = mybir.ActivationFunctionType
ALU = mybir.AluOpType
PI = math.pi


# ---------------------------------------------------------------- host prep
def _f16s(x):
    hi = np.float16(x)
    lo = np.float16(np.float32(x) - np.float32(hi))
    return hi, lo


def _f16_split(x):
    hi = x.astype(np.float16)
    lo = (x.astype(np.float32) - hi.astype(np.float32)).astype(np.float16)
    return hi.astype(np.float32), lo.astype(np.float32)


def _host_geometry(params, start_loc, surf_dist_lut, alpha_grid, beta_grid):
    params = params.astype(np.float64)
    alpha, beta, offset, shank = (params[:, 0], params[:, 1],
                                  params[:, 2], params[:, 3])
    a = alpha * _DEG2RAD
    b = beta * _DEG2RAD
    ca, sa = np.cos(a), np.sin(a)
    cb, sb = np.cos(b), np.sin(b)
    Bn = params.shape[0]
    Rx = np.zeros((Bn, 3, 3)); Ry = np.zeros((Bn, 3, 3))
    Rx[:, 0, 0] = 1; Rx[:, 1, 1] = ca; Rx[:, 1, 2] = -sa
    Rx[:, 2, 1] = sa; Rx[:, 2, 2] = ca
    Ry[:, 0, 0] = cb; Ry[:, 0, 2] = sb; Ry[:, 1, 1] = 1
    Ry[:, 2, 0] = -sb; Ry[:, 2, 2] = cb
    R = Rx @ Ry
    direction = np.einsum('bij,j->bi', R, np.array([0.0, 0.0, -1.0]))
    direction = direction / np.linalg.norm(direction, axis=-1, keepdims=True)
    lut = surf_dist_lut.astype(np.float64)
    na, nb = lut.shape
    ag, bg = alpha_grid.astype(np.float64), beta_grid.astype(np.float64)
    a_norm = 2.0 * (alpha - ag[0]) / (ag[-1] - ag[0] + 1e-08) - 1.0
    b_norm = 2.0 * (beta - bg[0]) / (bg[-1] - bg[0] + 1e-08) - 1.0
    ai = np.clip((a_norm + 1.0) * 0.5 * (na - 1), 0.0, na - 1.0)
    bi = np.clip((b_norm + 1.0) * 0.5 * (nb - 1), 0.0, nb - 1.0)
    a0 = np.clip(np.floor(ai), 0, na - 1).astype(np.int64)
    b0 = np.clip(np.floor(bi), 0, nb - 1).astype(np.int64)
    a1 = np.minimum(a0 + 1, na - 1)
    b1 = np.minimum(b0 + 1, nb - 1)
    fa = ai - a0
    fb = bi - b0
    v00 = lut[a0, b0]; v01 = lut[a0, b1]; v10 = lut[a1, b0]; v11 = lut[a1, b1]
    surf = (v00 * (1 - fa) * (1 - fb) + v01 * (1 - fa) * fb
            + v10 * fa * (1 - fb) + v11 * fa * fb)
    surf = np.maximum(surf, 1.0)
    penetration = surf - shank / 2.0 - offset
    grid_center = (start_loc.astype(np.float64)[None, :]
                   + direction * penetration[:, None])
    return grid_center, R, direction, shank


def _voxel_keep(v1_pos, grid_center, axis_dir, half_len):
    d = v1_pos.astype(np.float64) - grid_center[None, :]
    t = np.clip(d @ axis_dir, -half_len, half_len)
    dist = np.linalg.norm(d - t[:, None] * axis_dir[None, :], axis=1)
    return dist <= (CUT + XY_RAD + 0.5)


def _prep_core(gc_b, R_b, shank_b, logits_b, v1_pos_k, v1_prf_k, VP):
    """Per-core device input arrays for the lattice-factorized kernel."""
    Vk = v1_pos_k.shape[0]
    w = np.zeros((VP, 3))
    w[:Vk] = (v1_pos_k.astype(np.float64) - gc_b[None, :]) @ R_b
    wf = w.astype(np.float32)
    wh, wl = _f16_split(wf)
    bxy = (-0.5 * (w[:, 0] ** 2 + w[:, 1] ** 2)).astype(np.float32)
    bz = (-0.5 * w[:, 2] ** 2).astype(np.float32)
    bxy[Vk:] = -30000.0
    bz[Vk:] = -30000.0
    bxyh, bxyl = _f16_split(bxy)
    bzh, bzl = _f16_split(bz)
    onesv = np.ones(VP, np.float32)
    vt = np.stack([wh[:, 0], wh[:, 1], wl[:, 0], wl[:, 1], wh[:, 0],
                   wh[:, 1], onesv, onesv, bxyh, bxyl,
                   wh[:, 2], wl[:, 2], wh[:, 2], onesv, onesv, bzh, bzl],
                  axis=0).astype(np.float16)

    xs = np.arange(10) * 0.4 - 1.8
    zs = (np.linspace(0.0, 1.0, 10) - 0.5) * float(shank_b)
    cols = np.zeros((17, NXY + 10), np.float32)
    for ij in range(NXY):
        if ij < 100:
            iy, ix = ij // 10, ij % 10
            x, y = xs[ix], xs[iy]
            xh, xl = _f16s(x)
            yh, yl = _f16s(y)
            axyh, axyl = _f16s(-0.5 * (x * x + y * y))
            cols[0:10, ij] = [xh, yh, xh, yh, xl, yl, axyh, axyl, 1.0, 1.0]
        else:
            cols[6, ij] = -30000.0     # dummy xy slot -> Wxy = 0
            cols[8, ij] = 1.0
    for k in range(10):
        z = zs[k]
        zh, zl = _f16s(z)
        azh, azl = _f16s(-0.5 * z * z)
        cols[10:17, NXY + k] = [zh, zh, zl, azh, azl, 1.0, 1.0]
    rhs = cols.astype(np.float16)

    nch = VP // 128
    e3 = np.zeros((VP, 3), np.float32)
    e3[:Vk, 0] = v1_prf_k[:, 0]
    e3[:Vk, 1] = v1_prf_k[:, 1]
    e3[:Vk, 2] = 1.0
    e3t = np.ascontiguousarray(
        e3.reshape(nch, 128, 3).transpose(1, 0, 2).reshape(128, 3 * nch))

    lgt = np.full((NXY, NCC), -30.0, np.float32)
    iy, ix = np.divmod(np.arange(100), 10)
    for k in range(NCC):
        lgt[:100, k] = logits_b[iy * 100 + ix * 10 + k]

    # basis for the PE-generated u-coordinate megas: u[c,(k,i)] =
    # nych[k,c] + nycl[k,c] + iof[i]  (K=65 fp16 matmul, exact to fp32;
    # transposed center rows land at partition offsets 0 and 32)
    bas = np.zeros((65, 2 * NCC * WN), np.float16)
    iofv = np.tile(np.arange(W0, W0 + WN, dtype=np.float16), NCC)
    for j in range(NCC):
        bas[j, j * WN:(j + 1) * WN] = 1.0            # row-hi indicator
        bas[10 + j, j * WN:(j + 1) * WN] = 1.0       # row-lo indicator
        off = NCC * WN
        bas[32 + j, off + j * WN:off + (j + 1) * WN] = 1.0   # col-hi
        bas[42 + j, off + j * WN:off + (j + 1) * WN] = 1.0   # col-lo
    bas[32, 0:NCC * WN] = iofv      # iof for the K=33 row-factor matmuls
    bas[64, NCC * WN:] = iofv       # iof for the K=65 col-factor matmuls
    return {"vt": vt, "rhs": rhs, "e3": e3t,
            "lgt": np.ascontiguousarray(lgt),
            "eye": np.eye(128, dtype=np.float32),
            "bas": np.ascontiguousarray(bas)}


# ------------------------------------------------------------- device kernel
def _split_multiwaits(nc):
    """This walrus build accepts at most ONE sync wait per instruction.
    Tile emits several.  Engine instruction streams execute in order, so
    moving all but one wait onto single-wait NoOps inserted just before
    the instruction preserves semantics exactly."""
    cnt = 0
    for fn in nc.m.functions:
        for blk in fn.blocks:
            out = []
            for inst in blk.instructions:
                si = inst.sync_info
                if si is not None and si.on_wait is not None \
                        and len(si.on_wait) > 1:
                    waits = list(si.on_wait)
                    for w in waits[:-1]:
                        cnt += 1
                        out.append(mybir.InstNoOp(
                            name=f"WSPLIT-{cnt}",
                            engine=inst.engine,
                            ins=[], outs=[],
                            sync_info=mybir.SyncInfo(on_wait=[w],
                                                     on_update=[]),
                        ))
                    inst.sync_info = mybir.SyncInfo(
                        on_wait=[waits[-1]], on_update=list(si.on_update))
                out.append(inst)
            blk.instructions = out
    return cnt


def _build_nc(VP, n_warm=5):
    nch = VP // 128
    NL = NXY + 10    # 138 lattice columns
    UW = NCC * WN    # 960 u-mega columns per factor
    nc = bass.Bass()
    vt_d = nc.dram_tensor("vt", [17, VP], f16, kind="ExternalInput")
    rhs_d = nc.dram_tensor("rhs", [17, NL], f16, kind="ExternalInput")
    e3_d = nc.dram_tensor("e3", [128, 3 * nch], f32, kind="ExternalInput")
    lgt_d = nc.dram_tensor("lgt", [NXY, NCC], f32, kind="ExternalInput")
    eye_d = nc.dram_tensor("eye", [128, 128], f32, kind="ExternalInput")
    bas_d = nc.dram_tensor("bas", [65, 2 * UW], f16, kind="ExternalInput")
    out_d = nc.dram_tensor("out", [MAP_SIZE, MAP_SIZE], f32,
                           kind="ExternalOutput")

    with ExitStack() as ctx:
        tc = ctx.enter_context(tile.TileContext(nc))
        constp = ctx.enter_context(tc.tile_pool(name="const", bufs=1))
        parm = ctx.enter_context(tc.tile_pool(name="parm", bufs=1))
        work = ctx.enter_context(tc.tile_pool(name="work", bufs=6))
        psA = ctx.enter_context(
            tc.tile_pool(name="psA", bufs=1, space=bass.MemorySpace.PSUM))

        # Warmups first (top scheduler priority): ACT table load + PE ramp
        # burst run during the sem-init + input-DMA window.  The warmup
        # matmuls write into the (not-yet-used) uR PSUM tile.
        scr = constp.tile([1, 1], f32, tag="scr", name="scr")
        nc.vector.memset(scr[:], 0.0)
        nc.scalar.activation(scr[:], scr[:], AF.Exp, bias=0.0, scale=1.0)
        scr2 = constp.tile([1, 1], f32, tag="scr2", name="scr2")
        wrm = constp.tile([128, 512], f16, tag="wrm", name="wrm")
        nc.vector.memset(wrm[:], 0.0)
        dwrm = constp.tile([128, 384], f16, tag="dwrm", name="dwrm")
        nc.vector.memset(dwrm[:], 0.0)
        H = UW // 2
        uR0 = psA.tile([128, H], f32, tag="uR0", name="uR0")
        uR1 = psA.tile([128, H], f32, tag="uR1", name="uR1")
        uC0 = psA.tile([128, H], f32, tag="uC0", name="uC0")
        uC1 = psA.tile([128, H], f32, tag="uC1", name="uC1")
        for _ in range(n_warm):
            nc.tensor.matmul(uR0[:], wrm[:, 0:128], wrm[:, 0:H],
                             start=True, stop=True, skip_group_check=True)

        # input DMAs spread over 4 queues; vt (the phase-1 gate) split in 2
        vt_t = constp.tile([17, VP], f16, tag="vt", name="vt")
        vh = (VP // 256) * 128
        nc.sync.dma_start(vt_t[:, 0:vh], vt_d[:, 0:vh])
        nc.scalar.dma_start(vt_t[:, vh:VP], vt_d[:, vh:VP])
        rhs_t = constp.tile([17, NL], f16, tag="rhs", name="rhs")
        nc.gpsimd.dma_start(rhs_t[:], rhs_d[:])
        e3_t = constp.tile([128, 3 * nch], f32, tag="e3", name="e3")
        nc.scalar.dma_start(e3_t[:], e3_d[:])
        lg_t = constp.tile([NXY, NCC], f32, tag="lgt", name="lgt")
        nc.sync.dma_start(lg_t[:], lgt_d[:])
        eye_t = constp.tile([128, 128], f32, tag="eye", name="eye")
        nc.gpsimd.dma_start(eye_t[:], eye_d[:])
        bas_t = constp.tile([65, 2 * UW], f16, tag="bas", name="bas")
        nc.gpsimd.dma_start(bas_t[:], bas_d[:])

        ones_t = constp.tile([1, 128], f32, tag="ones", name="ones")
        nc.vector.memset(ones_t[:], 1.0)
        eye16 = constp.tile([128, 128], f16, tag="eye16", name="eye16")
        nc.vector.tensor_copy(eye16[:], eye_t[:])
        nytR = constp.tile([33, 128], f16, tag="nytR", name="nytR")
        nc.vector.memset(nytR[:], 0.0)
        nc.vector.memset(nytR[32:33, :], 1.0)
        nytC = constp.tile([65, 128], f16, tag="nytC", name="nytC")
        nc.vector.memset(nytC[:], 0.0)
        nc.vector.memset(nytC[64:65, :], 1.0)

        # zero tile for the top/bottom borders (DMA'd immediately); the
        # middle rows go out at the end via the full-width o_full tile.
        z_t = constp.tile([128, MAP_SIZE], f32, tag="z", name="z")
        nc.gpsimd.memset(z_t[:], 0.0)
        nc.sync.dma_start(out_d[0:W0, :], z_t[0:W0, :])
        nc.gpsimd.dma_start(out_d[W0 + WN:MAP_SIZE, :], z_t[0:W0, :])
        o_full = constp.tile([WN, MAP_SIZE], f32, tag="of", name="of")
        nc.vector.memset(o_full[:], 0.0)

        # sigmoid(logits) is independent of phase 1 -- run it early.
        en = parm.tile([128, NCC], f32, tag="en", name="en")
        nc.scalar.activation(en[:], lg_t[:], AF.Exp, bias=0.0, scale=-1.0)
        nc.vector.tensor_scalar_add(en[:], en[:], 1.0)
        pb = parm.tile([128, NCC], f32, tag="pb", name="pb")
        nc.vector.reciprocal(pb[:], en[:])

        # ---------------- phase 1: factorized soft match ----------------
        # Chunks are processed in pairs: two crosses land in one wide PSUM
        # tile, ONE exp covers both, one 4D-broadcast DVE op forms both
        # WzE blocks in fp16, and B accumulates via two fp16 matmuls per
        # chunk (wx split hi/lo on Pool; fp16 products are exact in the
        # fp32 PSUM accumulator, the fp16 rounding of WzE itself averages
        # out across voxels in the weighted-sum ratios).
        B_ps = psA.tile([128, 3 * NCC], f32, tag="B", name="B")
        with tc.tile_pool(name="psW", bufs=3,
                          space=bass.MemorySpace.PSUM) as psW:
            for p in range(nch // 2):
                ct2 = psW.tile([128, 2 * NL], f32, tag="cross",
                               name="cross")
                nc.tensor.matmul(ct2[:, 0:NL],
                                 vt_t[:, (2 * p) * 128:(2 * p + 1) * 128],
                                 rhs_t[:], start=True, stop=True)
                nc.tensor.matmul(
                    ct2[:, NL:2 * NL],
                    vt_t[:, (2 * p + 1) * 128:(2 * p + 2) * 128],
                    rhs_t[:], start=True, stop=True)
                wx2 = work.tile([128, 2 * NL], f32, tag="wx", name="wx")
                nc.scalar.activation(wx2[:], ct2[:], AF.Exp,
                                     bias=0.0, scale=EXP_SCALE)
                wze2 = work.tile([128, 6 * NCC], f16, tag="wze",
                                 name="wze")
                e3b = e3_t[:, 6 * p:6 * p + 6] \
                    .rearrange("p (c one f) -> p c one f", c=2, one=1) \
                    .broadcast_to([128, 2, NCC, 3])
                wzb = wx2[:].rearrange("p (c n) -> p c n", n=NL) \
                    [:, :, NXY:NL] \
                    .rearrange("p c (k one) -> p c k one", one=1) \
                    .broadcast_to([128, 2, NCC, 3])
                nc.vector.tensor_tensor(
                    wze2[:].rearrange("p (c k f) -> p c k f", k=NCC, f=3),
                    e3b, wzb, ALU.mult)
                for c in range(2):
                    k = 2 * p + c
                    wxh = work.tile([128, NXY], f16, tag="wxh", name="wxh")
                    nc.gpsimd.tensor_copy(
                        wxh[:], wx2[:, c * NL:c * NL + NXY])
                    wxl = work.tile([128, NXY], f16, tag="wxl", name="wxl")
                    nc.gpsimd.tensor_tensor(
                        wxl[:], wx2[:, c * NL:c * NL + NXY], wxh[:],
                        ALU.subtract)
                    wz3 = wze2[:, c * 3 * NCC:(c + 1) * 3 * NCC]
                    nc.tensor.matmul(B_ps[:], wxh[:], wz3,
                                     start=(k == 0), stop=False)
                    nc.tensor.matmul(B_ps[:], wxl[:], wz3,
                                     start=False, stop=(k == nch - 1))

        bsb = parm.tile([128, 3 * NCC], f32, tag="bsb", name="bsb")
        nc.vector.tensor_copy(bsb[:], B_ps[:])
        bs3 = bsb[:].rearrange("p (k f) -> p k f", f=3)

        with tc.tile_pool(name="psM", bufs=1,
                          space=bass.MemorySpace.PSUM) as psM:
            def pt(tag):
                return parm.tile([128, NCC], f32, tag=tag, name=tag)

            # ---------------- per-contact params (all DVE) ----------------
            # phos_size == 1 identically (KSIG*m_inv <= 0.447 on ecc in
            # [0,12]) so there is no sigma chain at all.
            t0 = pt("t0")
            nc.vector.tensor_scalar_add(t0[:], bs3[:, :, 2], 1e-8)
            rws = pt("rws"); nc.vector.reciprocal(rws[:], t0[:])
            pol = pt("pol")
            nc.vector.tensor_mul(pol[:], bs3[:, :, 0], rws[:])
            ecc = pt("ecc")
            nc.gpsimd.tensor_mul(ecc[:], bs3[:, :, 1], rws[:])

            # t20 = [u | pi/2 - |u|], u = pol*DEG2RAD - pi in [-pi, pi);
            # sin(u) = -sin(theta), sin(pi/2-|u|) = cos(u) = -cos(theta).
            # The hardware Sin spline is used; its trig table load hides in
            # the params window (ACT idle), as does the erf_derivative load
            # right after it.
            t20 = parm.tile([128, 2 * NCC], f32, tag="t20", name="t20")
            nc.vector.tensor_scalar(t20[:, 0:NCC], pol[:], _DEG2RAD, -PI,
                                    ALU.mult, ALU.add)
            nc.vector.scalar_tensor_tensor(t20[:, NCC:2 * NCC],
                                           t20[:, 0:NCC], -1.0,
                                           t20[:, 0:NCC],
                                           ALU.mult, ALU.max)   # |u|
            nc.vector.tensor_scalar(t20[:, NCC:2 * NCC],
                                    t20[:, NCC:2 * NCC], -1.0, PI / 2.0,
                                    ALU.mult, ALU.add)
            sc20 = parm.tile([128, 2 * NCC], f32, tag="sc20", name="sc20")
            nc.scalar.activation(sc20[:], t20[:], AF.Sin)
            sn = sc20[:, 0:NCC]          # -sin(theta)
            cs = sc20[:, NCC:2 * NCC]    # -cos(theta)

            t1 = pt("t1"); nc.vector.tensor_mul(t1[:], ecc[:], cs)
            nyc = pt("nyc")
            nc.vector.tensor_scalar(nyc[:], t1[:], -SE, -127.0,
                                    ALU.mult, ALU.add)
            t2 = pt("t2"); nc.vector.tensor_mul(t2[:], ecc[:], sn)
            nxc = pt("nxc")
            nc.vector.tensor_scalar(nxc[:], t2[:], SE, -128.0,
                                    ALU.mult, ALU.add)

            val = pt("val")
            nc.gpsimd.tensor_scalar_min(val[:], bs3[:, :, 2], 1.0)
            wc = pt("wc"); nc.gpsimd.tensor_mul(wc[:], pb[:], val[:])
            wch = parm.tile([128, NCC], f16, tag="wch", name="wch")
            nc.gpsimd.tensor_copy(wch[:], wc[:])

            # keep the PE pstate up through the params window; the dwrm
            # corner write makes these depend on bsb so the scheduler
            # cannot hoist them into phase 1.
            nc.gpsimd.tensor_copy(dwrm[0:1, 0:1], bsb[0:1, 0:1])
            for _ in range(5):
                nc.tensor.matmul(uR0[:, 0:384], wrm[:, 0:128],
                                 dwrm[:], start=True, stop=True,
                                 skip_group_check=True)

            # hi/lo fp16 split of the centers (so the PE basis matmuls that
            # generate u are exact to fp32); Y path first so the row-factor
            # pipeline starts as early as possible.
            nyHL = parm.tile([128, 2 * NCC], f16, tag="nyHL", name="nyHL")
            nc.vector.tensor_copy(nyHL[:, 0:NCC], nyc[:])
            nc.vector.tensor_tensor(nyHL[:, NCC:2 * NCC], nyc[:],
                                    nyHL[:, 0:NCC], ALU.subtract)
            nxHL = parm.tile([128, 2 * NCC], f16, tag="nxHL", name="nxHL")
            nc.vector.tensor_copy(nxHL[:, 0:NCC], nxc[:])
            nc.vector.tensor_tensor(nxHL[:, NCC:2 * NCC], nxc[:],
                                    nxHL[:, 0:NCC], ALU.subtract)

            # ---------------- phase 2: separable splat ----------------
            # u megas [128 contacts, 10 layers x 96 window] are produced by
            # K<=65 fp16 basis matmuls; Derivative_Erf(u) = (2/sqrt(pi))
            # exp(-u^2) reads PSUM directly (the constant, 4/pi after both
            # factors, cancels in the max-normalization).
            tpu = psM.tile([52, 128], f16, tag="tpu", name="tpu")
            nc.tensor.transpose(tpu[0:2 * NCC, 0:128], nyHL[:], eye16[:])
            nc.vector.tensor_copy(nytR[0:2 * NCC, :], tpu[0:2 * NCC, :])
            nc.tensor.matmul(uR0[:], nytR[:], bas_t[0:33, 0:H],
                             start=True, stop=True)
            nc.tensor.matmul(uR1[:], nytR[:], bas_t[0:33, H:UW],
                             start=True, stop=True)
            nc.tensor.transpose(tpu[32:32 + 2 * NCC, 0:128], nxHL[:],
                                eye16[:])
            nc.vector.tensor_copy(nytC[32:32 + 2 * NCC, :],
                                  tpu[32:32 + 2 * NCC, :])
            nc.tensor.matmul(uC0[:], nytC[:], bas_t[:, UW:UW + H],
                             start=True, stop=True)
            nc.tensor.matmul(uC1[:], nytC[:], bas_t[:, UW + H:2 * UW],
                             start=True, stop=True)
            yy = work.tile([128, UW], f16, tag="yy", name="yy")
            xx = work.tile([128, UW], f16, tag="xx", name="xx")
            nc.scalar.activation(yy[:, 0:H], uR0[:], AF.Derivative_Erf)
            nc.scalar.activation(xx[:, 0:H], uC0[:], AF.Derivative_Erf)
            nc.scalar.activation(yy[:, H:UW], uR1[:], AF.Derivative_Erf)
            nc.scalar.activation(xx[:, H:UW], uC1[:], AF.Derivative_Erf)

            def b3(t, k0, kn):   # [128, kn] -> [128, kn, WN] bcast
                return t[:, k0:k0 + kn] \
                    .rearrange("p (k one) -> p k one", one=1) \
                    .broadcast_to([128, kn, WN])

            yyw = work.tile([128, UW], f16, tag="yyw", name="yyw")
            nc.vector.tensor_tensor(
                yyw[:, 0:H].rearrange("p (k w) -> p k w", w=WN),
                yy[:, 0:H].rearrange("p (k w) -> p k w", w=WN),
                b3(wch, 0, NCC // 2), ALU.mult)
            nc.vector.tensor_tensor(
                yyw[:, H:UW].rearrange("p (k w) -> p k w", w=WN),
                yy[:, H:UW].rearrange("p (k w) -> p k w", w=WN),
                b3(wch, NCC // 2, NCC // 2), ALU.mult)

            mp = psM.tile([WN, WN], f32, tag="map", name="map")
            for k in range(NCC):
                o = k * WN
                nc.tensor.matmul(mp[:], yyw[:, o:o + WN], xx[:, o:o + WN],
                                 start=(k == 0), stop=(k == NCC - 1))

            # ---------------- normalize + store ----------------
            mx = parm.tile([WN, 1], f32, tag="mx", name="mx")
            nc.vector.reduce_max(mx[:], mp[:], axis=mybir.AxisListType.X)
            tpf = psM.tile([WN, 128], f32, tag="tpf", name="tpf")
            nc.tensor.transpose(tpf[0:1, 0:WN], mx[:], eye_t[0:WN, 0:WN])
            gm = parm.tile([1, 1], f32, tag="gm", name="gm")
            nc.vector.reduce_max(gm[:], tpf[0:1, 0:WN],
                                 axis=mybir.AxisListType.X)
            nc.vector.tensor_scalar_add(gm[:], gm[:], 1e-8)
            gi = parm.tile([1, 1], f32, tag="gi", name="gi")
            nc.vector.reciprocal(gi[:], gm[:])
            nc.tensor.matmul(tpf[0:WN, 127:128], ones_t[:, 0:WN], gi[:],
                             start=True, stop=True)
            gs = parm.tile([WN, 1], f32, tag="gs", name="gs")
            nc.vector.tensor_copy(gs[:], tpf[0:WN, 127:128])

            HW2 = 64
            nc.vector.tensor_scalar_mul(o_full[0:HW2, W0:W0 + WN],
                                        mp[0:HW2, :], gs[0:HW2, :])
            nc.sync.dma_start(out_d[W0:W0 + HW2, :], o_full[0:HW2, :])
            nc.vector.tensor_scalar_mul(o_full[HW2:WN, W0:W0 + WN],
                                        mp[HW2:WN, :], gs[HW2:WN, :])
            nc.scalar.dma_start(out_d[W0 + HW2:W0 + WN, :],
                                o_full[HW2:WN, :])
    return nc


# ----------------------------------------------------------------- entry
def _run(inputs, trace=False):
    params = np.asarray(inputs["params"], np.float32)
    logits = np.asarray(inputs["electrode_logits"], np.float32)
    v1_pos = np.asarray(inputs["v1_pos"], np.float32)
    v1_prf = np.asarray(inputs["v1_prf"], np.float32)
    start_loc = np.asarray(inputs["start_loc"], np.float32)
    surf_dist_lut = np.asarray(inputs["surf_dist_lut"], np.float32)
    alpha_grid = np.asarray(inputs["alpha_grid"], np.float32)
    beta_grid = np.asarray(inputs["beta_grid"], np.float32)

    gc, R, direction, shank = _host_geometry(
        params, start_loc, surf_dist_lut, alpha_grid, beta_grid)
    keeps = [_voxel_keep(v1_pos, gc[b], R[b, :, 2], shank[b] / 2.0)
             for b in range(B)]
    nkeep = max(int(k.sum()) for k in keeps)
    VP = max(256, ((nkeep + 255) // 256) * 256)

    in_maps = []
    for b in range(B):
        k = keeps[b]
        in_maps.append(_prep_core(gc[b], R[b], shank[b], logits[b],
                                  v1_pos[k], v1_prf[k], VP))
    nc = _build_nc(VP)
    _split_multiwaits(nc)
    res = run_bass_kernel_spmd(nc, in_maps, list(range(B)), trace=trace)
    out = np.stack([res.results[i]["out"] for i in range(B)])
    return out[:, None, :, :].astype(np.float32), res


def kernel(**inputs) -> np.ndarray:
    out, _ = _run(inputs, trace=False)
    return out


# revision 28
# speedup vs baseline: 1.2139x; 1.2139x over previous
"""Trainium2 Bass kernel for nn_DifferentiableSimulator.

Strategy (8 NeuronCores, B=8): one batch element per core, no collectives.

Host side (cheap, O(V+N)):
  - per-batch probe geometry: rotation, LUT bilinear interp (tiny)
  - per-batch voxel relevance sharding: keep voxels within CUT of the
    shank axis segment (+ probe radius).  Dropped voxels have weights
    <= e^{-CUT^2/4.5} relative to any weight that can influence an
    output pixel; empirically the output matches the dense reference
    to well under the harness tolerance.
  - lattice factorization: the 1000 contacts are a rigid 10x10x10 grid,
    so in the rotated frame  d2[n,v] = (x_i-wx_v)^2 + (y_j-wy_v)^2 +
    (z_k-wz_v)^2  with w = R^T (v - grid_center).  The soft-match weight
    matrix factorizes as W[n,v] = Wxy[(ij),v] * Wz[k,v]: only 110 gaussian
    columns per voxel instead of 1000.  Host ships the voxel features
    (fp16 hi/lo pairs so the fp16 matmul is ~fp32-exact: fp16 products are
    exact in the fp32 PSUM accumulator) and the 138 lattice columns.
  - contacts are reindexed m = k*128 + (iy*10+ix)  (28 dummy xy slots per
    z-layer with weight 0) so the per-z-layer weighted sums land exactly
    in contact-chunk layout with no transposes.

Device side (per core), phase 1 -- soft PRF match per 128-voxel chunk:
  one K=17 fp16 matmul -> xy/z gaussian exponents [128v, 138] in PSUM;
  ACT exp -> [Wxy | Wz] fp32; one DVE op forms WzE = Wz x [pol, ecc, 1]
  (broadcast APs); one fp32 matmul accumulates B[128ij, 30] =
  sum_v Wxy^T (Wz*E)  = all weighted sums, already contact-major.

Phase 2 -- separable splat, 96x96 window:
  phos_size = max(KSIG*m_inv, 1.0) == 1.0 identically for the reachable
  ecc range [0, 12] (KSIG*m_inv <= 0.447), so every gaussian has sigma
  exactly 1 px.  All splat centers lie in 128 +- 34.1 px (ecc <= 12),
  so only the [80,176) x [80,176) window of the 256x256 map is nonzero
  (>=12.9 px margin -> exp(-166), flushes to 0 in fp32 exactly like the
  reference).  sin/cos via a degree-9 odd polynomial on the DVE (no ACT
  table swaps; the only ACT table used is the Exp set).  Row factors
  (DVE) and col factors (Pool) are built as fp16 mega-tiles for 5 layers
  at a time, exp'd in two big ACT ops, weighted by wc via a broadcast
  multiply, and accumulated by 10 fp16 matmuls (K=128 contacts, N=96)
  into one [96,96] PSUM map; global max via the PE-transpose trick;
  scale; DMA the window out (borders are DMA'd from a zero tile early).

A PE warmup burst runs during startup to ramp the PE clock toward
2.4 GHz before phase 1.
"""
import math
from contextlib import ExitStack

import numpy as np

import concourse.bass as bass
import concourse.mybir as mybir
from concourse import tile
from concourse.bass_utils import run_bass_kernel_spmd

# ---- constants (must match the reference) ----
_CMAG_A = 0.75
_CMAG_B = 120.0
_CMAG_K = 17.3
_DEG2RAD = math.pi / 180.0
AMP = 100.0
_SPREAD = math.sqrt(AMP / 675.0)
VIEW_ANGLE = 90.0
MAP_SIZE = 256
SOFT_MATCH_SIGMA = 1.5

B = 8
NCC = 10                  # contact chunks = z-layers
NXY = 128                 # xy-lattice slots per layer (100 real + 28 dummy)
CUT = 7.0
XY_RAD = 1.8 * math.sqrt(2.0)
SE = MAP_SIZE / VIEW_ANGLE
KSIG = _SPREAD / 2.0 * SE
EXP_SCALE = 2.0 / (2.0 * SOFT_MATCH_SIGMA ** 2)   # 2/4.5
W0 = 80                   # splat window start (rows and cols)
WN = 96                   # splat window size

# sin(y) ~ y*(c0 + c1 y^2 + ... + c4 y^8) on [-pi, pi]; max err 1.7e-5
SIN_C = (9.99984590e-01, -1.66632589e-01, 8.31238590e-03,
         -1.93162309e-04, 2.17323611e-06)

f32 = mybir.dt.float32
f16 = mybir.dt.float16
i32 = mybir.dt.int32
AF = mybir.ActivationFunctionType
ALU = mybir.AluOpType
PI = math.pi


# ---------------------------------------------------------------- host prep
def _f16s(x):
    hi = np.float16(x)
    lo = np.float16(np.float32(x) - np.float32(hi))
    return hi, lo


def _f16_split(x):
    hi = x.astype(np.float16)
    lo = (x.astype(np.float32) - hi.astype(np.float32)).astype(np.float16)
    return hi.astype(np.float32), lo.astype(np.float32)


def _host_geometry(params, start_loc, surf_dist_lut, alpha_grid, beta_grid):
    params = params.astype(np.float64)
    alpha, beta, offset, shank = (params[:, 0], params[:, 1],
                                  params[:, 2], params[:, 3])
    a = alpha * _DEG2RAD
    b = beta * _DEG2RAD
    ca, sa = np.cos(a), np.sin(a)
    cb, sb = np.cos(b), np.sin(b)
    Bn = params.shape[0]
    Rx = np.zeros((Bn, 3, 3)); Ry = np.zeros((Bn, 3, 3))
    Rx[:, 0, 0] = 1; Rx[:, 1, 1] = ca; Rx[:, 1, 2] = -sa
    Rx[:, 2, 1] = sa; Rx[:, 2, 2] = ca
    Ry[:, 0, 0] = cb; Ry[:, 0, 2] = sb; Ry[:, 1, 1] = 1
    Ry[:, 2, 0] = -sb; Ry[:, 2, 2] = cb
    R = Rx @ Ry
    direction = np.einsum('bij,j->bi', R, np.array([0.0, 0.0, -1.0]))
    direction = direction / np.linalg.norm(direction, axis=-1, keepdims=True)
    lut = surf_dist_lut.astype(np.float64)
    na, nb = lut.shape
    ag, bg = alpha_grid.astype(np.float64), beta_grid.astype(np.float64)
    a_norm = 2.0 * (alpha - ag[0]) / (ag[-1] - ag[0] + 1e-08) - 1.0
    b_norm = 2.0 * (beta - bg[0]) / (bg[-1] - bg[0] + 1e-08) - 1.0
    ai = np.clip((a_norm + 1.0) * 0.5 * (na - 1), 0.0, na - 1.0)
    bi = np.clip((b_norm + 1.0) * 0.5 * (nb - 1), 0.0, nb - 1.0)
    a0 = np.clip(np.floor(ai), 0, na - 1).astype(np.int64)
    b0 = np.clip(np.floor(bi), 0, nb - 1).astype(np.int64)
    a1 = np.minimum(a0 + 1, na - 1)
    b1 = np.minimum(b0 + 1, nb - 1)
    fa = ai - a0
    fb = bi - b0
    v00 = lut[a0, b0]; v01 = lut[a0, b1]; v10 = lut[a1, b0]; v11 = lut[a1, b1]
    surf = (v00 * (1 - fa) * (1 - fb) + v01 * (1 - fa) * fb
            + v10 * fa * (1 - fb) + v11 * fa * fb)
    surf = np.maximum(surf, 1.0)
    penetration = surf - shank / 2.0 - offset
    grid_center = (start_loc.astype(np.float64)[None, :]
                   + direction * penetration[:, None])
    return grid_center, R, direction, shank


def _voxel_keep(v1_pos, grid_center, axis_dir, half_len):
    d = v1_pos.astype(np.float64) - grid_center[None, :]
    t = np.clip(d @ axis_dir, -half_len, half_len)
    dist = np.linalg.norm(d - t[:, None] * axis_dir[None, :], axis=1)
    return dist <= (CUT + XY_RAD + 0.5)


def _prep_core(gc_b, R_b, shank_b, logits_b, v1_pos_k, v1_prf_k, VP):
    """Per-core device input arrays for the lattice-factorized kernel."""
    Vk = v1_pos_k.shape[0]
    w = np.zeros((VP, 3))
    w[:Vk] = (v1_pos_k.astype(np.float64) - gc_b[None, :]) @ R_b
    wf = w.astype(np.float32)
    wh, wl = _f16_split(wf)
    bxy = (-0.5 * (w[:, 0] ** 2 + w[:, 1] ** 2)).astype(np.float32)
    bz = (-0.5 * w[:, 2] ** 2).astype(np.float32)
    bxy[Vk:] = -30000.0
    bz[Vk:] = -30000.0
    bxyh, bxyl = _f16_split(bxy)
    bzh, bzl = _f16_split(bz)
    onesv = np.ones(VP, np.float32)
    vt = np.stack([wh[:, 0], wh[:, 1], wl[:, 0], wl[:, 1], wh[:, 0],
                   wh[:, 1], onesv, onesv, bxyh, bxyl,
                   wh[:, 2], wl[:, 2], wh[:, 2], onesv, onesv, bzh, bzl],
                  axis=0).astype(np.float16)

    xs = np.arange(10) * 0.4 - 1.8
    zs = (np.linspace(0.0, 1.0, 10) - 0.5) * float(shank_b)
    cols = np.zeros((17, NXY + 10), np.float32)
    for ij in range(NXY):
        if ij < 100:
            iy, ix = ij // 10, ij % 10
            x, y = xs[ix], xs[iy]
            xh, xl = _f16s(x)
            yh, yl = _f16s(y)
            axyh, axyl = _f16s(-0.5 * (x * x + y * y))
            cols[0:10, ij] = [xh, yh, xh, yh, xl, yl, axyh, axyl, 1.0, 1.0]
        else:
            cols[6, ij] = -30000.0     # dummy xy slot -> Wxy = 0
            cols[8, ij] = 1.0
    for k in range(10):
        z = zs[k]
        zh, zl = _f16s(z)
        azh, azl = _f16s(-0.5 * z * z)
        cols[10:17, NXY + k] = [zh, zh, zl, azh, azl, 1.0, 1.0]
    rhs = cols.astype(np.float16)

    nch = VP // 128
    e3 = np.zeros((VP, 3), np.float32)
    e3[:Vk, 0] = v1_prf_k[:, 0]
    e3[:Vk, 1] = v1_prf_k[:, 1]
    e3[:Vk, 2] = 1.0
    e3t = np.ascontiguousarray(
        e3.reshape(nch, 128, 3).transpose(1, 0, 2).reshape(128, 3 * nch))

    lgt = np.full((NXY, NCC), -30.0, np.float32)
    iy, ix = np.divmod(np.arange(100), 10)
    for k in range(NCC):
        lgt[:100, k] = logits_b[iy * 100 + ix * 10 + k]

    # basis for the PE-generated u-coordinate megas: u[c,(k,i)] =
    # nych[k,c] + nycl[k,c] + iof[i]  (K=65 fp16 matmul, exact to fp32;
    # transposed center rows land at partition offsets 0 and 32)
    bas = np.zeros((65, 2 * NCC * WN), np.float16)
    iofv = np.tile(np.arange(W0, W0 + WN, dtype=np.float16), NCC)
    for j in range(NCC):
        bas[j, j * WN:(j + 1) * WN] = 1.0            # row-hi indicator
        bas[10 + j, j * WN:(j + 1) * WN] = 1.0       # row-lo indicator
        off = NCC * WN
        bas[32 + j, off + j * WN:off + (j + 1) * WN] = 1.0   # col-hi
        bas[42 + j, off + j * WN:off + (j + 1) * WN] = 1.0   # col-lo
    bas[32, 0:NCC * WN] = iofv      # iof for the K=33 row-factor matmuls
    bas[64, NCC * WN:] = iofv       # iof for the K=65 col-factor matmuls
    return {"vt": vt, "rhs": rhs, "e3": e3t,
            "lgt": np.ascontiguousarray(lgt),
            "eye": np.eye(128, dtype=np.float32),
            "bas": np.ascontiguousarray(bas)}


# ------------------------------------------------------------- device kernel
def _split_multiwaits(nc):
    """This walrus build accepts at most ONE sync wait per instruction.
    Tile emits several.  Engine instruction streams execute in order, so
    moving all but one wait onto single-wait NoOps inserted just before
    the instruction preserves semantics exactly."""
    cnt = 0
    for fn in nc.m.functions:
        for blk in fn.blocks:
            out = []
            for inst in blk.instructions:
                si = inst.sync_info
                if si is not None and si.on_wait is not None \
                        and len(si.on_wait) > 1:
                    waits = list(si.on_wait)
                    for w in waits[:-1]:
                        cnt += 1
                        out.append(mybir.InstNoOp(
                            name=f"WSPLIT-{cnt}",
                            engine=inst.engine,
                            ins=[], outs=[],
                            sync_info=mybir.SyncInfo(on_wait=[w],
                                                     on_update=[]),
                        ))
                    inst.sync_info = mybir.SyncInfo(
                        on_wait=[waits[-1]], on_update=list(si.on_update))
                out.append(inst)
            blk.instructions = out
    return cnt


def _build_nc(VP, n_warm=5):
    nch = VP // 128
    NL = NXY + 10    # 138 lattice columns
    UW = NCC * WN    # 960 u-mega columns per factor
    nc = bass.Bass()
    vt_d = nc.dram_tensor("vt", [17, VP], f16, kind="ExternalInput")
    rhs_d = nc.dram_tensor("rhs", [17, NL], f16, kind="ExternalInput")
    e3_d = nc.dram_tensor("e3", [128, 3 * nch], f32, kind="ExternalInput")
    lgt_d = nc.dram_tensor("lgt", [NXY, NCC], f32, kind="ExternalInput")
    eye_d = nc.dram_tensor("eye", [128, 128], f32, kind="ExternalInput")
    bas_d = nc.dram_tensor("bas", [65, 2 * UW], f16, kind="ExternalInput")
    out_d = nc.dram_tensor("out", [MAP_SIZE, MAP_SIZE], f32,
                           kind="ExternalOutput")

    with ExitStack() as ctx:
        tc = ctx.enter_context(tile.TileContext(nc))
        constp = ctx.enter_context(tc.tile_pool(name="const", bufs=1))
        parm = ctx.enter_context(tc.tile_pool(name="parm", bufs=1))
        work = ctx.enter_context(tc.tile_pool(name="work", bufs=6))
        psA = ctx.enter_context(
            tc.tile_pool(name="psA", bufs=1, space=bass.MemorySpace.PSUM))

        # Warmups first (top scheduler priority): ACT table load + PE ramp
        # burst run during the sem-init + input-DMA window.  The warmup
        # matmuls write into the (not-yet-used) uR PSUM tile.
        scr = constp.tile([1, 1], f32, tag="scr", name="scr")
        nc.vector.memset(scr[:], 0.0)
        nc.scalar.activation(scr[:], scr[:], AF.Exp, bias=0.0, scale=1.0)
        scr2 = constp.tile([1, 1], f32, tag="scr2", name="scr2")
        wrm = constp.tile([128, 512], f16, tag="wrm", name="wrm")
        nc.vector.memset(wrm[:], 0.0)
        dwrm = constp.tile([128, 384], f16, tag="dwrm", name="dwrm")
        nc.vector.memset(dwrm[:], 0.0)
        H = UW // 2
        uR0 = psA.tile([128, H], f32, tag="uR0", name="uR0")
        uR1 = psA.tile([128, H], f32, tag="uR1", name="uR1")
        uC0 = psA.tile([128, H], f32, tag="uC0", name="uC0")
        uC1 = psA.tile([128, H], f32, tag="uC1", name="uC1")
        for _ in range(n_warm):
            nc.tensor.matmul(uR0[:], wrm[:, 0:128], wrm[:, 0:H],
                             start=True, stop=True, skip_group_check=True)

        # input DMAs spread over 4 queues; vt (the phase-1 gate) split in 2
        vt_t = constp.tile([17, VP], f16, tag="vt", name="vt")
        vh = (VP // 256) * 128
        nc.sync.dma_start(vt_t[:, 0:vh], vt_d[:, 0:vh])
        nc.scalar.dma_start(vt_t[:, vh:VP], vt_d[:, vh:VP])
        rhs_t = constp.tile([17, NL], f16, tag="rhs", name="rhs")
        nc.gpsimd.dma_start(rhs_t[:], rhs_d[:])
        e3_t = constp.tile([128, 3 * nch], f32, tag="e3", name="e3")
        nc.scalar.dma_start(e3_t[:], e3_d[:])
        lg_t = constp.tile([NXY, NCC], f32, tag="lgt", name="lgt")
        nc.sync.dma_start(lg_t[:], lgt_d[:])
        eye_t = constp.tile([128, 128], f32, tag="eye", name="eye")
        nc.gpsimd.dma_start(eye_t[:], eye_d[:])
        bas_t = constp.tile([65, 2 * UW], f16, tag="bas", name="bas")
        nc.gpsimd.dma_start(bas_t[:], bas_d[:])

        ones_t = constp.tile([1, 128], f32, tag="ones", name="ones")
        nc.vector.memset(ones_t[:], 1.0)
        eye16 = constp.tile([128, 128], f16, tag="eye16", name="eye16")
        nc.vector.tensor_copy(eye16[:], eye_t[:])
        nytR = constp.tile([33, 128], f16, tag="nytR", name="nytR")
        nc.vector.memset(nytR[:], 0.0)
        nc.vector.memset(nytR[32:33, :], 1.0)
        nytC = constp.tile([65, 128], f16, tag="nytC", name="nytC")
        nc.vector.memset(nytC[:], 0.0)
        nc.vector.memset(nytC[64:65, :], 1.0)

        # zero tile for the top/bottom borders (DMA'd immediately); the
        # middle rows go out at the end via the full-width o_full tile.
        z_t = constp.tile([128, MAP_SIZE], f32, tag="z", name="z")
        nc.gpsimd.memset(z_t[:], 0.0)
        nc.sync.dma_start(out_d[0:W0, :], z_t[0:W0, :])
        nc.gpsimd.dma_start(out_d[W0 + WN:MAP_SIZE, :], z_t[0:W0, :])
        o_full = constp.tile([WN, MAP_SIZE], f32, tag="of", name="of")
        nc.vector.memset(o_full[:], 0.0)

        # sigmoid(logits) is independent of phase 1 -- run it early.
        en = parm.tile([128, NCC], f32, tag="en", name="en")
        nc.scalar.activation(en[:], lg_t[:], AF.Exp, bias=0.0, scale=-1.0)
        nc.vector.tensor_scalar_add(en[:], en[:], 1.0)
        pb = parm.tile([128, NCC], f32, tag="pb", name="pb")
        nc.vector.reciprocal(pb[:], en[:])

        # ---------------- phase 1: factorized soft match ----------------
        # Chunks are processed in pairs: two crosses land in one wide PSUM
        # tile, ONE exp covers both, one 4D-broadcast DVE op forms both
        # WzE blocks in fp16, and B accumulates via two fp16 matmuls per
        # chunk (wx split hi/lo on Pool; fp16 products are exact in the
        # fp32 PSUM accumulator, the fp16 rounding of WzE itself averages
        # out across voxels in the weighted-sum ratios).
        B_ps = psA.tile([128, 3 * NCC], f32, tag="B", name="B")
        with tc.tile_pool(name="psW", bufs=3,
                          space=bass.MemorySpace.PSUM) as psW:
            for p in range(nch // 2):
                ct2 = psW.tile([128, 2 * NL], f32, tag="cross",
                               name="cross")
                nc.tensor.matmul(ct2[:, 0:NL],
                                 vt_t[:, (2 * p) * 128:(2 * p + 1) * 128],
                                 rhs_t[:], start=True, stop=True)
                nc.tensor.matmul(
                    ct2[:, NL:2 * NL],
                    vt_t[:, (2 * p + 1) * 128:(2 * p + 2) * 128],
                    rhs_t[:], start=True, stop=True)
                wx2 = work.tile([128, 2 * NL], f16, tag="wx", name="wx")
                nc.scalar.activation(wx2[:], ct2[:], AF.Exp,
                                     bias=0.0, scale=EXP_SCALE)
                wze2 = work.tile([128, 6 * NCC], f16, tag="wze",
                                 name="wze")
                e3b = e3_t[:, 6 * p:6 * p + 6] \
                    .rearrange("p (c one f) -> p c one f", c=2, one=1) \
                    .broadcast_to([128, 2, NCC, 3])
                wzb = wx2[:].rearrange("p (c n) -> p c n", n=NL) \
                    [:, :, NXY:NL] \
                    .rearrange("p c (k one) -> p c k one", one=1) \
                    .broadcast_to([128, 2, NCC, 3])
                nc.vector.tensor_tensor(
                    wze2[:].rearrange("p (c k f) -> p c k f", k=NCC, f=3),
                    e3b, wzb, ALU.mult)
                for c in range(2):
                    k = 2 * p + c
                    nc.tensor.matmul(B_ps[:],
                                     wx2[:, c * NL:c * NL + NXY],
                                     wze2[:, c * 3 * NCC:(c + 1) * 3 * NCC],
                                     start=(k == 0), stop=(k == nch - 1))

        bsb = parm.tile([128, 3 * NCC], f32, tag="bsb", name="bsb")
        nc.vector.tensor_copy(bsb[:], B_ps[:])
        bs3 = bsb[:].rearrange("p (k f) -> p k f", f=3)

        with tc.tile_pool(name="psM", bufs=1,
                          space=bass.MemorySpace.PSUM) as psM:
            def pt(tag):
                return parm.tile([128, NCC], f32, tag=tag, name=tag)

            # ---------------- per-contact params (all DVE) ----------------
            # phos_size == 1 identically (KSIG*m_inv <= 0.447 on ecc in
            # [0,12]) so there is no sigma chain at all.
            t0 = pt("t0")
            nc.vector.tensor_scalar_add(t0[:], bs3[:, :, 2], 1e-8)
            rws = pt("rws"); nc.vector.reciprocal(rws[:], t0[:])
            pol = pt("pol")
            nc.vector.tensor_mul(pol[:], bs3[:, :, 0], rws[:])
            ecc = pt("ecc")
            nc.gpsimd.tensor_mul(ecc[:], bs3[:, :, 1], rws[:])

            # t20 = [u | pi/2 - |u|], u = pol*DEG2RAD - pi in [-pi, pi);
            # sin(u) = -sin(theta), sin(pi/2-|u|) = cos(u) = -cos(theta).
            # The hardware Sin spline is used; its trig table load hides in
            # the params window (ACT idle), as does the erf_derivative load
            # right after it.
            t20 = parm.tile([128, 2 * NCC], f32, tag="t20", name="t20")
            nc.vector.tensor_scalar(t20[:, 0:NCC], pol[:], _DEG2RAD, -PI,
                                    ALU.mult, ALU.add)
            nc.vector.scalar_tensor_tensor(t20[:, NCC:2 * NCC],
                                           t20[:, 0:NCC], -1.0,
                                           t20[:, 0:NCC],
                                           ALU.mult, ALU.max)   # |u|
            nc.vector.tensor_scalar(t20[:, NCC:2 * NCC],
                                    t20[:, NCC:2 * NCC], -1.0, PI / 2.0,
                                    ALU.mult, ALU.add)
            sc20 = parm.tile([128, 2 * NCC], f32, tag="sc20", name="sc20")
            nc.scalar.activation(sc20[:], t20[:], AF.Sin)
            sn = sc20[:, 0:NCC]          # -sin(theta)
            cs = sc20[:, NCC:2 * NCC]    # -cos(theta)

            t1 = pt("t1"); nc.vector.tensor_mul(t1[:], ecc[:], cs)
            nyc = pt("nyc")
            nc.vector.tensor_scalar(nyc[:], t1[:], -SE, -127.0,
                                    ALU.mult, ALU.add)
            t2 = pt("t2"); nc.vector.tensor_mul(t2[:], ecc[:], sn)
            nxc = pt("nxc")
            nc.vector.tensor_scalar(nxc[:], t2[:], SE, -128.0,
                                    ALU.mult, ALU.add)

            val = pt("val")
            nc.gpsimd.tensor_scalar_min(val[:], bs3[:, :, 2], 1.0)
            wc = pt("wc"); nc.gpsimd.tensor_mul(wc[:], pb[:], val[:])
            wch = parm.tile([128, NCC], f16, tag="wch", name="wch")
            nc.gpsimd.tensor_copy(wch[:], wc[:])


            # hi/lo fp16 split of the centers (so the PE basis matmuls that
            # generate u are exact to fp32); Y path first so the row-factor
            # pipeline starts as early as possible.
            nyHL = parm.tile([128, 2 * NCC], f16, tag="nyHL", name="nyHL")
            nc.vector.tensor_copy(nyHL[:, 0:NCC], nyc[:])
            nc.vector.tensor_tensor(nyHL[:, NCC:2 * NCC], nyc[:],
                                    nyHL[:, 0:NCC], ALU.subtract)
            nxHL = parm.tile([128, 2 * NCC], f16, tag="nxHL", name="nxHL")
            nc.vector.tensor_copy(nxHL[:, 0:NCC], nxc[:])
            nc.vector.tensor_tensor(nxHL[:, NCC:2 * NCC], nxc[:],
                                    nxHL[:, 0:NCC], ALU.subtract)

            # ---------------- phase 2: separable splat ----------------
            # u megas [128 contacts, 10 layers x 96 window] are produced by
            # K<=65 fp16 basis matmuls; Derivative_Erf(u) = (2/sqrt(pi))
            # exp(-u^2) reads PSUM directly (the constant, 4/pi after both
            # factors, cancels in the max-normalization).
            tpu = psM.tile([52, 128], f16, tag="tpu", name="tpu")
            nc.tensor.transpose(tpu[0:2 * NCC, 0:128], nyHL[:], eye16[:])
            nc.vector.tensor_copy(nytR[0:2 * NCC, :], tpu[0:2 * NCC, :])
            nc.tensor.matmul(uR0[:], nytR[:], bas_t[0:33, 0:H],
                             start=True, stop=True)
            nc.tensor.matmul(uR1[:], nytR[:], bas_t[0:33, H:UW],
                             start=True, stop=True)
            nc.tensor.transpose(tpu[32:32 + 2 * NCC, 0:128], nxHL[:],
                                eye16[:])
            nc.vector.tensor_copy(nytC[32:32 + 2 * NCC, :],
                                  tpu[32:32 + 2 * NCC, :])
            nc.tensor.matmul(uC0[:], nytC[:], bas_t[:, UW:UW + H],
                             start=True, stop=True)
            nc.tensor.matmul(uC1[:], nytC[:], bas_t[:, UW + H:2 * UW],
                             start=True, stop=True)
            yy = work.tile([128, UW], f16, tag="yy", name="yy")
            xx = work.tile([128, UW], f16, tag="xx", name="xx")
            nc.scalar.activation(yy[:, 0:H], uR0[:], AF.Derivative_Erf)
            nc.scalar.activation(xx[:, 0:H], uC0[:], AF.Derivative_Erf)
            nc.scalar.activation(yy[:, H:UW], uR1[:], AF.Derivative_Erf)
            nc.scalar.activation(xx[:, H:UW], uC1[:], AF.Derivative_Erf)

            def b3(t, k0, kn):   # [128, kn] -> [128, kn, WN] bcast
                return t[:, k0:k0 + kn] \
                    .rearrange("p (k one) -> p k one", one=1) \
                    .broadcast_to([128, kn, WN])

            yyw = work.tile([128, UW], f16, tag="yyw", name="yyw")
            nc.vector.tensor_tensor(
                yyw[:, 0:H].rearrange("p (k w) -> p k w", w=WN),
                yy[:, 0:H].rearrange("p (k w) -> p k w", w=WN),
                b3(wch, 0, NCC // 2), ALU.mult)
            nc.vector.tensor_tensor(
                yyw[:, H:UW].rearrange("p (k w) -> p k w", w=WN),
                yy[:, H:UW].rearrange("p (k w) -> p k w", w=WN),
                b3(wch, NCC // 2, NCC // 2), ALU.mult)

            mp = psM.tile([WN, WN], f32, tag="map", name="map")
            for k in range(NCC):
                o = k * WN
                nc.tensor.matmul(mp[:], yyw[:, o:o + WN], xx[:, o:o + WN],
                                 start=(k == 0), stop=(k == NCC - 1))

            # ---------------- normalize + store ----------------
            mx = parm.tile([WN, 1], f32, tag="mx", name="mx")
            nc.vector.reduce_max(mx[:], mp[:], axis=mybir.AxisListType.X)
            tpf = psM.tile([WN, 128], f32, tag="tpf", name="tpf")
            nc.tensor.transpose(tpf[0:1, 0:WN], mx[:], eye_t[0:WN, 0:WN])
            gm = parm.tile([1, 1], f32, tag="gm", name="gm")
            nc.vector.reduce_max(gm[:], tpf[0:1, 0:WN],
                                 axis=mybir.AxisListType.X)
            nc.vector.tensor_scalar_add(gm[:], gm[:], 1e-8)
            gi = parm.tile([1, 1], f32, tag="gi", name="gi")
            nc.vector.reciprocal(gi[:], gm[:])
            nc.tensor.matmul(tpf[0:WN, 127:128], ones_t[:, 0:WN], gi[:],
                             start=True, stop=True)
            gs = parm.tile([WN, 1], f32, tag="gs", name="gs")
            nc.vector.tensor_copy(gs[:], tpf[0:WN, 127:128])

            HW2 = 64
            nc.vector.tensor_scalar_mul(o_full[0:HW2, W0:W0 + WN],
                                        mp[0:HW2, :], gs[0:HW2, :])
            nc.sync.dma_start(out_d[W0:W0 + HW2, :], o_full[0:HW2, :])
            nc.vector.tensor_scalar_mul(o_full[HW2:WN, W0:W0 + WN],
                                        mp[HW2:WN, :], gs[HW2:WN, :])
            nc.scalar.dma_start(out_d[W0 + HW2:W0 + WN, :],
                                o_full[HW2:WN, :])
    return nc


# ----------------------------------------------------------------- entry
def _run(inputs, trace=False):
    params = np.asarray(inputs["params"], np.float32)
    logits = np.asarray(inputs["electrode_logits"], np.float32)
    v1_pos = np.asarray(inputs["v1_pos"], np.float32)
    v1_prf = np.asarray(inputs["v1_prf"], np.float32)
    start_loc = np.asarray(inputs["start_loc"], np.float32)
    surf_dist_lut = np.asarray(inputs["surf_dist_lut"], np.float32)
    alpha_grid = np.asarray(inputs["alpha_grid"], np.float32)
    beta_grid = np.asarray(inputs["beta_grid"], np.float32)

    gc, R, direction, shank = _host_geometry(
        params, start_loc, surf_dist_lut, alpha_grid, beta_grid)
    keeps = [_voxel_keep(v1_pos, gc[b], R[b, :, 2], shank[b] / 2.0)
             for b in range(B)]
    nkeep = max(int(k.sum()) for k in keeps)
    VP = max(256, ((nkeep + 255) // 256) * 256)

    in_maps = []
    for b in range(B):
        k = keeps[b]
        in_maps.append(_prep_core(gc[b], R[b], shank[b], logits[b],
                                  v1_pos[k], v1_prf[k], VP))
    nc = _build_nc(VP)
    _split_multiwaits(nc)
    res = run_bass_kernel_spmd(nc, in_maps, list(range(B)), trace=trace)
    out = np.stack([res.results[i]["out"] for i in range(B)])
    return out[:, None, :, :].astype(np.float32), res


def kernel(**inputs) -> np.ndarray:
    out, _ = _run(inputs, trace=False)
    return out
